# revision 19
# baseline (speedup 1.0000x reference)
"""CapsNet dynamic-routing kernel for 8 TRN2 NeuronCores.

Problem: x [256,1152,8], W [1152,10,8,16], 3 routing iterations, out [256,10,16,1].

Strategy (sharded over the input-capsule axis I, 144 capsules / core):
  u_hat is NEVER materialized (it would be 189MB). Instead each routing
  iteration computes, per core (ID = 144*8 = 1152 local (i,d) rows):
    s_partial[b,(o,e)] = x_flat[b,:] @ (c*W)_flat[:,(o,e)]     (dense matmul, K=ID)
    s = AllReduce(s_partial) over the 8 I-shards                (164KB, 1/iter)
    v = squash(s)            (computed redundantly on all cores)
    G[(i,d),(o,e)] = x_flat^T @ v_flat                          (dense matmul)
    agree[i,o] = (1/B) * sum_{d,e} (W_flat * G)[(i,d),(o,e)]    (local, no comm)
    b += agree ; c = softmax(b, axis=o)                         (local)
  Iteration 1 uses uniform c=1/O folded into the squash scale, so the
  c*W build is skipped there.  The last iteration skips the agreement.
  Matmul operands are bf16 (fp32 matmul runs 2 passes at 1/4 rate on
  TRN2); all accumulation stays fp32 (PSUM), the AllReduce is fp32.
  Softmax skips max-subtraction: |b| stays O(1) for this routing.
"""

import numpy as np

B, I, O, DIN, DOUT = 256, 1152, 10, 8, 16
NCORES = 8
I_SH = I // NCORES          # 144 input capsules per core
ID = I_SH * DIN             # 1152 local (i,d) rows
NT = ID // 128              # 9 partition tiles of (i,d)
BT = B // 128               # 2 partition tiles of batch
OE = O * DOUT               # 160
ROUTING_ITERS = 3

_CACHE = {}


def _bc(ap_mod, ap, n):
    """View an AP with an extra innermost broadcast axis of length n."""
    return ap_mod.AP(tensor=ap.tensor, offset=ap.offset, ap=[*ap.ap, [0, n]])


def _build():
    import concourse.bass as bass
    import concourse.bacc as bacc
    import concourse.tile as tile
    from concourse import mybir

    f32 = mybir.dt.float32
    bf16 = mybir.dt.bfloat16
    AF = mybir.ActivationFunctionType
    ALU = mybir.AluOpType

    nc = bacc.Bacc("TRN2", target_bir_lowering=False, debug=False,
                   num_devices=NCORES)

    # All inputs pre-tiled on host to [128, ...] so every DMA is contiguous.
    xT_d = nc.dram_tensor("xT", [128, NT, B], bf16, kind="ExternalInput")
    xf_d = nc.dram_tensor("xf", [128, BT, ID], bf16, kind="ExternalInput")
    Wb_d = nc.dram_tensor("Wb", [128, NT, OE], bf16, kind="ExternalInput")
    MB_d = nc.dram_tensor("Mblk", [128, 128], bf16, kind="ExternalInput")
    out_d = nc.dram_tensor("out", [B, OE], f32, kind="ExternalOutput")

    with tile.TileContext(nc) as tc:
        with (
            tc.tile_pool(name="sb", bufs=1) as sb,
            tc.tile_pool(name="work", bufs=2) as work,
            tc.tile_pool(name="ps_s", bufs=2, space="PSUM") as ps_s,
            tc.tile_pool(name="ps_g", bufs=2, space="PSUM") as ps_g,
            tc.tile_pool(name="ps_a", bufs=2, space="PSUM") as ps_a,
            tc.tile_pool(name="dram", bufs=3, space="DRAM") as dram,
        ):
            # ---- persistent SBUF tensors ----
            xT = sb.tile([128, NT, B], bf16)      # x_flat^T tiles (lhsT for s)
            xf = sb.tile([128, BT, ID], bf16)     # x_flat tiles (lhsT for G)
            Wb = sb.tile([128, NT, OE], bf16)     # W_flat bf16
            Mblk = sb.tile([128, 128], bf16)      # 8x8 block-diag ones
            bq = sb.tile([128, NT, O], f32)       # routing logits b (expanded)
            Wc = sb.tile([128, NT, OE], bf16)     # c * W
            s_sb = sb.tile([128, BT, OE], bf16)   # local partial s
            sf = sb.tile([128, BT, OE], bf16)     # all-reduced s
            vb = sb.tile([128, BT, OE], bf16)     # squash(s) bf16 (rhs for G)
            vf = sb.tile([128, BT, OE], f32)      # squash(s) f32 (final out)
            WG = sb.tile([128, NT, OE], bf16)     # W * G
            A1 = sb.tile([128, NT, O], bf16)      # e-reduced agreement
            dmy = sb.tile([128, 1], f32)          # ACT table-prefetch scratch

            # Warm-up collective: the first collective of an execution pays
            # tens of us of ncfw wake latency plus any cross-core start
            # skew.  Issue a tiny AllReduce whose input chain has no real
            # dependencies so its doorbell rings as early as possible and
            # that latency overlaps the input DMAs + first matmuls.
            warm = sb.tile([128, 4], f32)
            wcc_in = dram.tile([128, 4], f32, tag="w_in")
            wcc_out = dram.tile([128, 4], f32, tag="w_out",
                                addr_space="Shared")
            nc.vector.memset(warm[:], 0.0)
            nc.sync.dma_start(out=wcc_in[:], in_=warm[:])
            nc.gpsimd.collective_compute(
                "AllReduce", ALU.add,
                replica_groups=[list(range(NCORES))],
                ins=[wcc_in.opt()], outs=[wcc_out.opt()])
            nc.sync.dma_start(out=warm[:], in_=wcc_out[:])

            nc.sync.dma_start(out=Wb[:], in_=Wb_d[:])
            nc.sync.dma_start(out=xT[:], in_=xT_d[:])
            nc.sync.dma_start(out=Mblk[:], in_=MB_d[:])
            nc.sync.dma_start(out=xf[:], in_=xf_d[:])

            # Pre-load the Sqrt ACT table during setup (table loads are
            # ~1.3us each and otherwise land on the post-AllReduce chain).
            nc.scalar.activation(out=dmy[:], in_=warm[:, 0:1], func=AF.Sqrt)

            Wb4 = Wb.rearrange("p t (o e) -> p t o e", o=O)
            Wc4 = Wc.rearrange("p t (o e) -> p t o e", o=O)
            WG4 = WG.rearrange("p t (o e) -> p t o e", o=O)
            sf4 = sf.rearrange("p b (o e) -> p b o e", o=O)
            vb4 = vb.rearrange("p b (o e) -> p b o e", o=O)
            vf4 = vf.rearrange("p b (o e) -> p b o e", o=O)

            for it in range(ROUTING_ITERS):
                first, last = it == 0, it == ROUTING_ITERS - 1

                if first:
                    rhs = Wb      # uniform c = 1/O folded into squash scale
                else:
                    # c = softmax(b) over o per (i,d) row; |b| is O(1) so no
                    # max-subtraction is needed (matches jax softmax exactly
                    # up to rounding).
                    ex = work.tile([128, NT, O], f32, tag="ex")
                    nc.scalar.activation(out=ex[:], in_=bq[:], func=AF.Exp)
                    # prefetch Sqrt table for this iteration's squash; rides
                    # the s-matmul + AllReduce slack
                    nc.scalar.activation(out=dmy[:], in_=ex[:, 0, 0:1],
                                         func=AF.Sqrt)
                    sm = work.tile([128, NT], f32, tag="sm")
                    nc.vector.reduce_sum(out=sm[:], in_=ex[:],
                                         axis=mybir.AxisListType.X)
                    nc.vector.reciprocal(out=sm[:], in_=sm[:])
                    # Wc[t] = (exp(b)*recip_sum) * W, fused per k-tile so the
                    # first s-matmuls start while later tiles still build
                    for t in range(NT):
                        nc.vector.scalar_tensor_tensor(
                            out=Wc4[:, t], in0=_bc(bass, ex[:, t, :], DOUT),
                            scalar=sm[:, t:t + 1], in1=Wb4[:, t],
                            op0=ALU.mult, op1=ALU.mult)
                    rhs = Wc

                # s_partial = x_flat @ rhs : out [b-tile 128, OE]
                for bt in range(BT):
                    s_ps = ps_s.tile([128, OE], f32, tag="s_ps")
                    for k in range(NT):
                        nc.tensor.matmul(
                            s_ps[:],
                            xT[:, k, bt * 128:(bt + 1) * 128],
                            rhs[:, k, :],
                            start=(k == 0), stop=(k == NT - 1))
                    nc.vector.tensor_copy(s_sb[:, bt, :], s_ps[:])

                # AllReduce s over the 8 I-shards
                cc_in = dram.tile([BT, 128, OE], bf16, tag="cc_in")
                cc_out = dram.tile([BT, 128, OE], bf16, tag="cc_out",
                                   addr_space="Shared")
                nc.sync.dma_start(
                    out=cc_in.rearrange("b p f -> p b f"), in_=s_sb[:])
                nc.gpsimd.collective_compute(
                    "AllReduce", ALU.add,
                    replica_groups=[list(range(NCORES))],
                    ins=[cc_in.opt()], outs=[cc_out.opt()])
                nc.sync.dma_start(
                    out=sf[:], in_=cc_out.rearrange("b p f -> p b f"))

                # squash: v = s * sqrt(ss)/(1+ss) per (b, o); iteration 1
                # carries c=1/O as s_raw = O*s_true.
                sq = work.tile([128, BT, OE], f32, tag="sq")
                nc.vector.tensor_tensor(out=sq[:], in0=sf[:], in1=sf[:],
                                        op=ALU.mult)
                ss = work.tile([128, BT, O], f32, tag="ss")
                nc.vector.reduce_sum(
                    out=ss[:], in_=sq.rearrange("p b (o e) -> p b o e", o=O),
                    axis=mybir.AxisListType.X)
                t1 = work.tile([128, BT, O], f32, tag="t1")
                nc.scalar.activation(out=t1[:], in_=ss[:], func=AF.Sqrt)
                den = work.tile([128, BT, O], f32, tag="den")
                if first:
                    # ss_raw = O^2*ss_true:
                    #   v = s_raw*(1/O^2)*sqrt(ss_raw)/(1+ss_raw/O^2)
                    nc.vector.tensor_scalar(
                        out=den[:], in0=ss[:], scalar1=1.0 / (O * O),
                        scalar2=1.0, op0=ALU.mult, op1=ALU.add)
                else:
                    nc.vector.tensor_scalar_add(den[:], ss[:], 1.0)
                nc.vector.reciprocal(out=den[:], in_=den[:])
                rat = work.tile([128, BT, O], f32, tag="rat")
                nc.vector.tensor_tensor(out=rat[:], in0=t1[:], in1=den[:],
                                        op=ALU.mult)
                if first:
                    nc.vector.tensor_scalar_mul(rat[:], rat[:], 1.0 / (O * O))
                if not last:
                    # prefetch Exp table for the next softmax; rides the
                    # agreement-path slack
                    nc.scalar.activation(out=dmy[:], in_=rat[:, 0, 0:1],
                                         func=AF.Exp)
                vout4 = vf4 if last else vb4
                nc.vector.tensor_tensor(
                    out=vout4[:], in0=sf4[:],
                    in1=_bc(bass, rat[:], DOUT), op=ALU.mult)

                if last:
                    nc.sync.dma_start(
                        out=out_d.rearrange("(b p) f -> p b f", p=128),
                        in_=vf[:])
                else:
                    # G = x_flat^T @ v ; agree = (1/B) sum_de W*G ; b += agree
                    for mt in range(NT):
                        g_ps = ps_g.tile([128, OE], f32, tag="g_ps")
                        for bt in range(BT):
                            nc.tensor.matmul(
                                g_ps[:],
                                xf[:, bt, mt * 128:(mt + 1) * 128],
                                vb[:, bt, :],
                                start=(bt == 0), stop=(bt == BT - 1))
                        nc.vector.tensor_tensor(
                            out=WG[:, mt, :], in0=Wb[:, mt, :], in1=g_ps[:],
                            op=ALU.mult)
                    with nc.allow_low_precision("agreement tolerates bf16"):
                        nc.vector.reduce_sum(out=A1[:], in_=WG4[:],
                                             axis=mybir.AxisListType.X)
                    # d-sums of all NT tiles land in one PSUM tile so the
                    # b-update is a single vector op
                    a_ps = ps_a.tile([128, NT, O], f32, tag="a_ps")
                    for mt in range(NT):
                        nc.tensor.matmul(a_ps[:, mt, :], Mblk[:],
                                         A1[:, mt, :], start=True, stop=True)
                    if first:
                        nc.vector.tensor_scalar_mul(bq[:], a_ps[:], 1.0 / B)
                    else:
                        nc.vector.scalar_tensor_tensor(
                            out=bq[:], in0=a_ps[:], scalar=1.0 / B,
                            in1=bq[:], op0=ALU.mult, op1=ALU.add)

    nc.compile()
    return nc


def _get_nc():
    if "nc" not in _CACHE:
        _CACHE["nc"] = _build()
    return _CACHE["nc"]


def _tile128(a):
    """[R, C] -> [128, R//128, C] with row r = t*128+p at [p, t]."""
    r, c = a.shape
    return np.ascontiguousarray(
        a.reshape(r // 128, 128, c).transpose(1, 0, 2))


def _make_in_maps(x, W):
    from concourse import mybir
    bfdt = mybir.dt.np(mybir.dt.bfloat16)
    x = np.asarray(x, dtype=np.float32)
    W = np.asarray(W, dtype=np.float32)
    mblk = np.kron(np.eye(16, dtype=np.float32),
                   np.ones((8, 8), dtype=np.float32)).astype(bfdt)
    in_maps = []
    for core in range(NCORES):
        isl = slice(core * I_SH, (core + 1) * I_SH)
        x_flat = x[:, isl, :].reshape(B, ID)
        w_flat = W[isl].transpose(0, 2, 1, 3).reshape(ID, OE)
        in_maps.append({
            "xT": _tile128(np.ascontiguousarray(x_flat.T)).astype(bfdt),
            "xf": _tile128(x_flat).astype(bfdt),
            "Wb": _tile128(w_flat).astype(bfdt),
            "Mblk": mblk,
        })
    return in_maps


def _ensure_ntff_hook():
    """This image's antenv lacks axon_hooks; reconstruct it so trace=True
    can reach the NTFF profiler in libaxon_pjrt.so."""
    import sys
    import types
    try:
        import antenv.axon_hooks  # noqa: F401
        return
    except ImportError:
        pass
    try:
        import antenv
        from trn_agent_boot.trn_boot import _ntff_profile_via_ctypes
        hook = _ntff_profile_via_ctypes("/opt/axon/libaxon_pjrt.so")
        mod = types.ModuleType("antenv.axon_hooks")
        mod._hook = hook
        mod.get_axon_ntff_profile_hook = lambda: mod._hook
        mod.set_axon_ntff_profile_hook = (
            lambda h: setattr(mod, "_hook", h))
        sys.modules["antenv.axon_hooks"] = mod
        antenv.axon_hooks = mod
    except Exception as e:  # profiling is best-effort
        print("ntff hook setup failed:", e)


def _run_hw(x, W, trace=False, **kwargs):
    from concourse import bass_utils
    if trace:
        _ensure_ntff_hook()
    nc = _get_nc()
    res = bass_utils.run_bass_kernel_spmd(
        nc, _make_in_maps(x, W), core_ids=list(range(NCORES)),
        trace=trace, **kwargs)
    out = res.results[0]["out"]
    return out.reshape(B, O, DOUT)[..., None].astype(np.float32), res


def kernel(x, W):
    out, _ = _run_hw(x, W, trace=False)
    return out


# revision 37
# speedup vs baseline: 1.4594x; 1.4594x over previous
"""CapsNet dynamic-routing kernel for 8 TRN2 NeuronCores.

Problem: x [256,1152,8], W [1152,10,8,16], 3 routing iterations, out [256,10,16,1].

Strategy (sharded over the input-capsule axis I, 144 capsules / core):
  u_hat is NEVER materialized (it would be 189MB). Instead each routing
  iteration computes, per core (ID = 144*8 = 1152 local (i,d) rows):
    s_partial[b,(o,e)] = x_flat[b,:] @ (c*W)_flat[:,(o,e)]     (dense matmul, K=ID)
    s = AllReduce(s_partial) over the 8 I-shards                (164KB, 1/iter)
    v = squash(s)            (computed redundantly on all cores)
    G[(i,d),(o,e)] = x_flat^T @ v_flat                          (dense matmul)
    agree[i,o] = (1/B) * sum_{d,e} (W_flat * G)[(i,d),(o,e)]    (local, no comm)
    b += agree ; c = softmax(b, axis=o)                         (local)
  Iteration 1 uses uniform c=1/O folded into the squash scale, so the
  c*W build is skipped there.  The last iteration skips the agreement.
  Matmul operands are bf16 (fp32 matmul runs 2 passes at 1/4 rate on
  TRN2); all accumulation stays fp32 (PSUM), the AllReduce is fp32.
  Softmax skips max-subtraction: |b| stays O(1) for this routing.
"""

import numpy as np

B, I, O, DIN, DOUT = 256, 1152, 10, 8, 16
NCORES = 8
I_SH = I // NCORES          # 144 input capsules per core
ID = I_SH * DIN             # 1152 local (i,d) rows
NT = ID // 128              # 9 partition tiles of (i,d)
BT = B // 128               # 2 partition tiles of batch
OE = O * DOUT               # 160
ROUTING_ITERS = 3

_CACHE = {}


def _bc(ap_mod, ap, n):
    """View an AP with an extra innermost broadcast axis of length n."""
    return ap_mod.AP(tensor=ap.tensor, offset=ap.offset, ap=[*ap.ap, [0, n]])


def _build():
    import concourse.bass as bass
    import concourse.bacc as bacc
    import concourse.tile as tile
    from concourse import mybir

    f32 = mybir.dt.float32
    bf16 = mybir.dt.bfloat16
    AF = mybir.ActivationFunctionType
    ALU = mybir.AluOpType

    nc = bacc.Bacc("TRN2", target_bir_lowering=False, debug=False,
                   num_devices=NCORES)

    # All inputs pre-tiled on host to [128, ...] so every DMA is contiguous.
    xT_d = nc.dram_tensor("xT", [128, NT, B], bf16, kind="ExternalInput")
    xf_d = nc.dram_tensor("xf", [128, BT, ID], bf16, kind="ExternalInput")
    Wb_d = nc.dram_tensor("Wb", [128, NT, OE], bf16, kind="ExternalInput")
    MB_d = nc.dram_tensor("Mblk", [128, 128], bf16, kind="ExternalInput")
    # final iteration uses ReduceScatter: each core emits its batch shard
    BSH = B // NCORES
    out_d = nc.dram_tensor("out", [BSH, OE], f32, kind="ExternalOutput")

    with tile.TileContext(nc) as tc:
        with (
            tc.tile_pool(name="sb", bufs=1) as sb,
            tc.tile_pool(name="work", bufs=2) as work,
            tc.tile_pool(name="ps_s", bufs=2, space="PSUM") as ps_s,
            tc.tile_pool(name="ps_g", bufs=2, space="PSUM") as ps_g,
            tc.tile_pool(name="ps_a", bufs=2, space="PSUM") as ps_a,
            tc.tile_pool(name="dram", bufs=3, space="DRAM") as dram,
        ):
            # ---- persistent SBUF tensors ----
            xT = sb.tile([128, NT, B], bf16)      # x_flat^T tiles (lhsT for s)
            xf = sb.tile([128, BT, ID], bf16)     # x_flat tiles (lhsT for G)
            Wb = sb.tile([128, NT, OE], bf16)     # W_flat bf16
            Mblk = sb.tile([128, 128], bf16)      # 8x8 block-diag ones
            bq = sb.tile([128, NT, O], f32)       # routing logits b (expanded)
            Wc = sb.tile([128, NT, OE], bf16)     # c * W
            s_sb = sb.tile([128, BT, OE], bf16)   # local partial s
            sf = sb.tile([128, BT, OE], bf16)     # all-reduced s
            vb = sb.tile([128, BT, OE], bf16)     # squash(s) bf16 (rhs for G)
            WG = sb.tile([128, NT, OE], bf16)     # W * G
            A1 = sb.tile([128, NT, O], bf16)      # e-reduced agreement
            dmy = sb.tile([128, 1], f32)          # ACT table-prefetch scratch

            # (Measured: the first collective's ncfw wake latency is a fixed
            # ~43us-from-execution-start (warm) regardless of when its
            # doorbell rings, so a warm-up collective only serializes in
            # front of the first real AllReduce — don't add one.)
            nc.sync.dma_start(out=Wb[:], in_=Wb_d[:])
            nc.sync.dma_start(out=xT[:], in_=xT_d[:])
            nc.sync.dma_start(out=Mblk[:], in_=MB_d[:])
            nc.gpsimd.dma_start(out=xf[:], in_=xf_d[:])

            # Pre-load the Sqrt ACT table during setup (table loads are
            # ~1.3us each and otherwise land on the post-AllReduce chain).
            nc.scalar.activation(out=dmy[:], in_=Mblk[:, 0:1], func=AF.Sqrt)

            Wb4 = Wb.rearrange("p t (o e) -> p t o e", o=O)
            Wc4 = Wc.rearrange("p t (o e) -> p t o e", o=O)
            WG4 = WG.rearrange("p t (o e) -> p t o e", o=O)
            sf4 = sf.rearrange("p b (o e) -> p b o e", o=O)
            vb4 = vb.rearrange("p b (o e) -> p b o e", o=O)

            for it in range(ROUTING_ITERS):
                first, last = it == 0, it == ROUTING_ITERS - 1

                if first:
                    rhs = Wb      # uniform c = 1/O folded into squash scale
                else:
                    # c = softmax(b) over o per (i,d) row; |b| is O(1) so no
                    # max-subtraction is needed (matches jax softmax exactly
                    # up to rounding).
                    ex = work.tile([128, NT, O], f32, tag="ex")
                    nc.scalar.activation(out=ex[:], in_=bq[:], func=AF.Exp)
                    # prefetch Sqrt table for this iteration's squash; rides
                    # the s-matmul + AllReduce slack
                    nc.scalar.activation(out=dmy[:], in_=ex[:, 0, 0:1],
                                         func=AF.Sqrt)
                    sm = work.tile([128, NT], f32, tag="sm")
                    nc.vector.reduce_sum(out=sm[:], in_=ex[:],
                                         axis=mybir.AxisListType.X)
                    nc.vector.reciprocal(out=sm[:], in_=sm[:])
                    nc.vector.tensor_tensor(
                        out=ex[:], in0=ex[:], in1=_bc(bass, sm[:], O),
                        op=ALU.mult)
                    # Wc = c * W in a few chunks so the first s-matmuls can
                    # start while later tiles still build
                    GRP = 3
                    for g in range(0, NT, GRP):
                        nc.vector.tensor_tensor(
                            out=Wc4[:, g:g + GRP],
                            in0=_bc(bass, ex[:, g:g + GRP, :], DOUT),
                            in1=Wb4[:, g:g + GRP], op=ALU.mult)
                    rhs = Wc

                # s_partial = x_flat @ rhs : out [b-tile 128, OE]
                cc_in = dram.tile([BT, 128, OE], bf16, tag="cc_in")
                for bt in range(BT):
                    s_ps = ps_s.tile([128, OE], f32, tag="s_ps")
                    for k in range(NT):
                        nc.tensor.matmul(
                            s_ps[:],
                            xT[:, k, bt * 128:(bt + 1) * 128],
                            rhs[:, k, :],
                            start=(k == 0), stop=(k == NT - 1))
                    nc.vector.tensor_copy(s_sb[:, bt, :], s_ps[:])
                    nc.sync.dma_start(out=cc_in[bt], in_=s_sb[:, bt, :])

                if last:
                    # Final iteration: each core only needs a batch shard of
                    # v for the output, so ReduceScatter (cheaper than
                    # AllReduce) and the host concatenates per-core shards.
                    cc_rs = dram.tile([BSH, OE], bf16, tag="cc_rs")
                    nc.gpsimd.collective_compute(
                        "ReduceScatter", ALU.add,
                        replica_groups=[list(range(NCORES))],
                        ins=[cc_in.opt()], outs=[cc_rs.opt()])
                    s3 = sb.tile([BSH, OE], bf16)
                    nc.sync.dma_start(out=s3[:], in_=cc_rs[:])
                    sq3 = work.tile([BSH, OE], f32, tag="sq3")
                    nc.vector.tensor_tensor(out=sq3[:], in0=s3[:],
                                            in1=s3[:], op=ALU.mult)
                    ss3 = work.tile([BSH, O], f32, tag="ss3")
                    nc.vector.reduce_sum(
                        out=ss3[:],
                        in_=sq3.rearrange("p (o e) -> p o e", o=O),
                        axis=mybir.AxisListType.X)
                    t13 = work.tile([BSH, O], f32, tag="t13")
                    nc.scalar.activation(out=t13[:], in_=ss3[:], func=AF.Sqrt)
                    den3 = work.tile([BSH, O], f32, tag="den3")
                    nc.vector.tensor_scalar_add(den3[:], ss3[:], 1.0)
                    nc.vector.reciprocal(out=den3[:], in_=den3[:])
                    rat3 = work.tile([BSH, O], f32, tag="rat3")
                    nc.vector.tensor_tensor(out=rat3[:], in0=t13[:],
                                            in1=den3[:], op=ALU.mult)
                    v3 = work.tile([BSH, OE], f32, tag="v3")
                    nc.vector.tensor_tensor(
                        out=v3.rearrange("p (o e) -> p o e", o=O),
                        in0=s3.rearrange("p (o e) -> p o e", o=O),
                        in1=_bc(bass, rat3[:], DOUT), op=ALU.mult)
                    nc.sync.dma_start(out=out_d[:], in_=v3[:])
                    continue

                # AllReduce s over the 8 I-shards
                cc_out = dram.tile([BT, 128, OE], bf16, tag="cc_out",
                                   addr_space="Shared")
                nc.gpsimd.collective_compute(
                    "AllReduce", ALU.add,
                    replica_groups=[list(range(NCORES))],
                    ins=[cc_in.opt()], outs=[cc_out.opt()])
                nc.sync.dma_start(
                    out=sf[:], in_=cc_out.rearrange("b p f -> p b f"))

                # squash: v = s * sqrt(ss)/(1+ss) per (b, o); iteration 1
                # carries c=1/O as s_raw = O*s_true.
                sq = work.tile([128, BT, OE], f32, tag="sq")
                nc.vector.tensor_tensor(out=sq[:], in0=sf[:], in1=sf[:],
                                        op=ALU.mult)
                ss = work.tile([128, BT, O], f32, tag="ss")
                nc.vector.reduce_sum(
                    out=ss[:], in_=sq.rearrange("p b (o e) -> p b o e", o=O),
                    axis=mybir.AxisListType.X)
                t1 = work.tile([128, BT, O], f32, tag="t1")
                nc.scalar.activation(out=t1[:], in_=ss[:], func=AF.Sqrt)
                den = work.tile([128, BT, O], f32, tag="den")
                if first:
                    # ss_raw = O^2*ss_true:
                    #   v = s_raw*(1/O^2)*sqrt(ss_raw)/(1+ss_raw/O^2)
                    nc.vector.tensor_scalar(
                        out=den[:], in0=ss[:], scalar1=1.0 / (O * O),
                        scalar2=1.0, op0=ALU.mult, op1=ALU.add)
                else:
                    nc.vector.tensor_scalar_add(den[:], ss[:], 1.0)
                nc.vector.reciprocal(out=den[:], in_=den[:])
                rat = work.tile([128, BT, O], f32, tag="rat")
                nc.vector.tensor_tensor(out=rat[:], in0=t1[:], in1=den[:],
                                        op=ALU.mult)
                if first:
                    nc.vector.tensor_scalar_mul(rat[:], rat[:], 1.0 / (O * O))
                # prefetch Exp table for the next softmax; rides the
                # agreement-path slack
                nc.scalar.activation(out=dmy[:], in_=rat[:, 0, 0:1],
                                     func=AF.Exp)
                nc.vector.tensor_tensor(
                    out=vb4[:], in0=sf4[:],
                    in1=_bc(bass, rat[:], DOUT), op=ALU.mult)

                # G = x_flat^T @ v ; agree = (1/B) sum_de W*G ; b += agree.
                # Three (i,d)-tiles share one PSUM bank (3*640B < 2KB) so
                # the W*G multiply and e-reduction run once per group.
                GW = 3
                for g in range(0, NT, GW):
                    g_ps = ps_g.tile([128, GW, OE], f32, tag="g_ps")
                    for j in range(GW):
                        for bt in range(BT):
                            nc.tensor.matmul(
                                g_ps[:, j, :],
                                xf[:, bt, (g + j) * 128:(g + j + 1) * 128],
                                vb[:, bt, :],
                                start=(bt == 0), stop=(bt == BT - 1))
                    nc.vector.tensor_tensor(
                        out=WG[:, g:g + GW, :], in0=Wb[:, g:g + GW, :],
                        in1=g_ps[:], op=ALU.mult)
                    with nc.allow_low_precision("agreement tolerates bf16"):
                        nc.vector.reduce_sum(
                            out=A1[:, g:g + GW, :],
                            in_=WG[:, g:g + GW, :].rearrange(
                                "p g (o e) -> p (g o) e", o=O),
                            axis=mybir.AxisListType.X)
                # d-sums of all NT tiles land in one PSUM tile so the
                # b-update is a single vector op
                a_ps = ps_a.tile([128, NT, O], f32, tag="a_ps")
                for mt in range(NT):
                    nc.tensor.matmul(a_ps[:, mt, :], Mblk[:],
                                     A1[:, mt, :], start=True, stop=True)
                if first:
                    nc.vector.tensor_scalar_mul(bq[:], a_ps[:], 1.0 / B)
                else:
                    nc.vector.scalar_tensor_tensor(
                        out=bq[:], in0=a_ps[:], scalar=1.0 / B,
                        in1=bq[:], op0=ALU.mult, op1=ALU.add)

    nc.compile()
    return nc


def _get_nc():
    if "nc" not in _CACHE:
        _CACHE["nc"] = _build()
    return _CACHE["nc"]


def _tile128(a):
    """[R, C] -> [128, R//128, C] with row r = t*128+p at [p, t]."""
    r, c = a.shape
    return np.ascontiguousarray(
        a.reshape(r // 128, 128, c).transpose(1, 0, 2))


def _make_in_maps(x, W):
    from concourse import mybir
    bfdt = mybir.dt.np(mybir.dt.bfloat16)
    x = np.asarray(x, dtype=np.float32)
    W = np.asarray(W, dtype=np.float32)
    mblk = np.kron(np.eye(16, dtype=np.float32),
                   np.ones((8, 8), dtype=np.float32)).astype(bfdt)
    in_maps = []
    for core in range(NCORES):
        isl = slice(core * I_SH, (core + 1) * I_SH)
        x_flat = x[:, isl, :].reshape(B, ID)
        w_flat = W[isl].transpose(0, 2, 1, 3).reshape(ID, OE)
        in_maps.append({
            "xT": _tile128(np.ascontiguousarray(x_flat.T)).astype(bfdt),
            "xf": _tile128(x_flat).astype(bfdt),
            "Wb": _tile128(w_flat).astype(bfdt),
            "Mblk": mblk,
        })
    return in_maps


def _ensure_ntff_hook():
    """This image's antenv lacks axon_hooks; reconstruct it so trace=True
    can reach the NTFF profiler in libaxon_pjrt.so."""
    import sys
    import types
    try:
        import antenv.axon_hooks  # noqa: F401
        return
    except ImportError:
        pass
    try:
        import antenv
        from trn_agent_boot.trn_boot import _ntff_profile_via_ctypes
        hook = _ntff_profile_via_ctypes("/opt/axon/libaxon_pjrt.so")
        mod = types.ModuleType("antenv.axon_hooks")
        mod._hook = hook
        mod.get_axon_ntff_profile_hook = lambda: mod._hook
        mod.set_axon_ntff_profile_hook = (
            lambda h: setattr(mod, "_hook", h))
        sys.modules["antenv.axon_hooks"] = mod
        antenv.axon_hooks = mod
    except Exception as e:  # profiling is best-effort
        print("ntff hook setup failed:", e)


def _run_hw(x, W, trace=False, **kwargs):
    from concourse import bass_utils
    if trace:
        _ensure_ntff_hook()
    nc = _get_nc()
    res = bass_utils.run_bass_kernel_spmd(
        nc, _make_in_maps(x, W), core_ids=list(range(NCORES)),
        trace=trace, **kwargs)
    out = np.concatenate([res.results[c]["out"] for c in range(NCORES)],
                         axis=0)
    return out.reshape(B, O, DOUT)[..., None].astype(np.float32), res


def kernel(x, W):
    out, _ = _run_hw(x, W, trace=False)
    return out


# revision 38
# speedup vs baseline: 1.5913x; 1.0903x over previous
"""CapsNet dynamic-routing kernel for 8 TRN2 NeuronCores.

Problem: x [256,1152,8], W [1152,10,8,16], 3 routing iterations, out [256,10,16,1].

Strategy (sharded over the input-capsule axis I, 144 capsules / core):
  u_hat is NEVER materialized (it would be 189MB). Instead each routing
  iteration computes, per core (ID = 144*8 = 1152 local (i,d) rows):
    s_partial[b,(o,e)] = x_flat[b,:] @ (c*W)_flat[:,(o,e)]     (dense matmul, K=ID)
    s = AllReduce(s_partial) over the 8 I-shards                (82KB bf16, 1/iter)
    v = squash(s)            (computed redundantly on all cores)
    G[(i,d),(o,e)] = x_flat^T @ v_flat                          (dense matmul)
    agree[i,o] = (1/B) * sum_{d,e} (W_flat * G)[(i,d),(o,e)]    (local, no comm)
    b += agree ; c = softmax(b, axis=o)                         (local)
  Iteration 1 uses uniform c=1/O folded into the squash scale, so the
  c*W build is skipped there.  The final iteration needs no agreement and
  only a batch shard of v per core, so it uses ReduceScatter instead of
  AllReduce and the host concatenates the 8 output shards.
  Matmul operands are bf16 (fp32 matmul runs 2 passes at 1/4 rate on
  TRN2); accumulation stays fp32 (PSUM); collectives carry bf16.
  Softmax skips max-subtraction: |b| stays O(1) for this routing.
"""

import numpy as np

B, I, O, DIN, DOUT = 256, 1152, 10, 8, 16
NCORES = 8
I_SH = I // NCORES          # 144 input capsules per core
ID = I_SH * DIN             # 1152 local (i,d) rows
NT = ID // 128              # 9 partition tiles of (i,d)
BT = B // 128               # 2 partition tiles of batch
OE = O * DOUT               # 160
ROUTING_ITERS = 3

_CACHE = {}


def _bc(ap_mod, ap, n):
    """View an AP with an extra innermost broadcast axis of length n."""
    return ap_mod.AP(tensor=ap.tensor, offset=ap.offset, ap=[*ap.ap, [0, n]])


def _build():
    import concourse.bass as bass
    import concourse.bacc as bacc
    import concourse.tile as tile
    from concourse import mybir

    f32 = mybir.dt.float32
    bf16 = mybir.dt.bfloat16
    AF = mybir.ActivationFunctionType
    ALU = mybir.AluOpType

    nc = bacc.Bacc("TRN2", target_bir_lowering=False, debug=False,
                   num_devices=NCORES)

    # All inputs pre-tiled on host to [128, ...] so every DMA is contiguous.
    xT_d = nc.dram_tensor("xT", [128, NT, B], bf16, kind="ExternalInput")
    xf_d = nc.dram_tensor("xf", [128, BT, ID], bf16, kind="ExternalInput")
    Wb_d = nc.dram_tensor("Wb", [128, NT, OE], bf16, kind="ExternalInput")
    MB_d = nc.dram_tensor("Mblk", [128, 128], bf16, kind="ExternalInput")
    # final iteration uses ReduceScatter: each core emits its batch shard
    BSH = B // NCORES
    out_d = nc.dram_tensor("out", [BSH, OE], f32, kind="ExternalOutput")

    with tile.TileContext(nc) as tc:
        with (
            tc.tile_pool(name="sb", bufs=1) as sb,
            tc.tile_pool(name="work", bufs=2) as work,
            tc.tile_pool(name="ps_s", bufs=2, space="PSUM") as ps_s,
            tc.tile_pool(name="ps_g", bufs=2, space="PSUM") as ps_g,
            tc.tile_pool(name="ps_a", bufs=2, space="PSUM") as ps_a,
            tc.tile_pool(name="dram", bufs=3, space="DRAM") as dram,
        ):
            # ---- persistent SBUF tensors ----
            xT = sb.tile([128, NT, B], bf16)      # x_flat^T tiles (lhsT for s)
            xf = sb.tile([128, BT, ID], bf16)     # x_flat tiles (lhsT for G)
            Wb = sb.tile([128, NT, OE], bf16)     # W_flat bf16
            Mblk = sb.tile([128, 128], bf16)      # 8x8 block-diag ones
            bq = sb.tile([128, NT, O], f32)       # routing logits b (expanded)
            Wc = sb.tile([128, NT, OE], bf16)     # c * W
            s_sb = sb.tile([128, BT, OE], bf16)   # local partial s
            sf = sb.tile([128, BT, OE], bf16)     # all-reduced s
            vb = sb.tile([128, BT, OE], bf16)     # squash(s) bf16 (rhs for G)
            WG = sb.tile([128, NT, OE], bf16)     # W * G
            A1 = sb.tile([128, NT, O], bf16)      # e-reduced agreement
            dmy = sb.tile([128, 1], f32)          # ACT table-prefetch scratch

            # (Measured: the first collective's ncfw wake latency is a fixed
            # ~43us-from-execution-start (warm) regardless of when its
            # doorbell rings, so a warm-up collective only serializes in
            # front of the first real AllReduce — don't add one.)
            nc.sync.dma_start(out=Wb[:], in_=Wb_d[:])
            nc.sync.dma_start(out=xT[:], in_=xT_d[:])
            nc.sync.dma_start(out=Mblk[:], in_=MB_d[:])
            nc.gpsimd.dma_start(out=xf[:], in_=xf_d[:])

            # Pre-load the Sqrt ACT table during setup (table loads are
            # ~1.3us each and otherwise land on the post-AllReduce chain).
            nc.scalar.activation(out=dmy[:], in_=Mblk[:, 0:1], func=AF.Sqrt)

            Wb4 = Wb.rearrange("p t (o e) -> p t o e", o=O)
            Wc4 = Wc.rearrange("p t (o e) -> p t o e", o=O)
            WG4 = WG.rearrange("p t (o e) -> p t o e", o=O)
            sf4 = sf.rearrange("p b (o e) -> p b o e", o=O)
            vb4 = vb.rearrange("p b (o e) -> p b o e", o=O)

            for it in range(ROUTING_ITERS):
                first, last = it == 0, it == ROUTING_ITERS - 1

                if first:
                    rhs = Wb      # uniform c = 1/O folded into squash scale
                else:
                    # c = softmax(b) over o per (i,d) row; |b| is O(1) so no
                    # max-subtraction is needed (matches jax softmax exactly
                    # up to rounding).
                    ex = work.tile([128, NT, O], f32, tag="ex")
                    nc.scalar.activation(out=ex[:], in_=bq[:], func=AF.Exp)
                    # prefetch Sqrt table for this iteration's squash; rides
                    # the s-matmul + AllReduce slack
                    nc.scalar.activation(out=dmy[:], in_=ex[:, 0, 0:1],
                                         func=AF.Sqrt)
                    sm = work.tile([128, NT], f32, tag="sm")
                    nc.vector.reduce_sum(out=sm[:], in_=ex[:],
                                         axis=mybir.AxisListType.X)
                    nc.vector.reciprocal(out=sm[:], in_=sm[:])
                    nc.vector.tensor_tensor(
                        out=ex[:], in0=ex[:], in1=_bc(bass, sm[:], O),
                        op=ALU.mult)
                    # Wc = c * W in a few chunks so the first s-matmuls can
                    # start while later tiles still build
                    GRP = 3
                    for g in range(0, NT, GRP):
                        nc.vector.tensor_tensor(
                            out=Wc4[:, g:g + GRP],
                            in0=_bc(bass, ex[:, g:g + GRP, :], DOUT),
                            in1=Wb4[:, g:g + GRP], op=ALU.mult)
                    rhs = Wc

                # s_partial = x_flat @ rhs : out [b-tile 128, OE]
                cc_in = dram.tile([BT, 128, OE], bf16, tag="cc_in")
                for bt in range(BT):
                    s_ps = ps_s.tile([128, OE], f32, tag="s_ps")
                    for k in range(NT):
                        nc.tensor.matmul(
                            s_ps[:],
                            xT[:, k, bt * 128:(bt + 1) * 128],
                            rhs[:, k, :],
                            start=(k == 0), stop=(k == NT - 1))
                    nc.vector.tensor_copy(s_sb[:, bt, :], s_ps[:])
                    nc.sync.dma_start(out=cc_in[bt], in_=s_sb[:, bt, :])

                if last:
                    # Final iteration: each core only needs a batch shard of
                    # v for the output, so ReduceScatter (cheaper than
                    # AllReduce) and the host concatenates per-core shards.
                    cc_rs = dram.tile([BSH, OE], bf16, tag="cc_rs")
                    nc.gpsimd.collective_compute(
                        "ReduceScatter", ALU.add,
                        replica_groups=[list(range(NCORES))],
                        ins=[cc_in.opt()], outs=[cc_rs.opt()])
                    s3 = sb.tile([BSH, OE], bf16)
                    nc.sync.dma_start(out=s3[:], in_=cc_rs[:])
                    sq3 = work.tile([BSH, OE], f32, tag="sq3")
                    nc.vector.tensor_tensor(out=sq3[:], in0=s3[:],
                                            in1=s3[:], op=ALU.mult)
                    ss3 = work.tile([BSH, O], f32, tag="ss3")
                    nc.vector.reduce_sum(
                        out=ss3[:],
                        in_=sq3.rearrange("p (o e) -> p o e", o=O),
                        axis=mybir.AxisListType.X)
                    t13 = work.tile([BSH, O], f32, tag="t13")
                    nc.scalar.activation(out=t13[:], in_=ss3[:], func=AF.Sqrt)
                    den3 = work.tile([BSH, O], f32, tag="den3")
                    nc.vector.tensor_scalar_add(den3[:], ss3[:], 1.0)
                    nc.vector.reciprocal(out=den3[:], in_=den3[:])
                    rat3 = work.tile([BSH, O], f32, tag="rat3")
                    nc.vector.tensor_tensor(out=rat3[:], in0=t13[:],
                                            in1=den3[:], op=ALU.mult)
                    v3 = work.tile([BSH, OE], f32, tag="v3")
                    nc.vector.tensor_tensor(
                        out=v3.rearrange("p (o e) -> p o e", o=O),
                        in0=s3.rearrange("p (o e) -> p o e", o=O),
                        in1=_bc(bass, rat3[:], DOUT), op=ALU.mult)
                    nc.sync.dma_start(out=out_d[:], in_=v3[:])
                    continue

                # AllReduce s over the 8 I-shards
                cc_out = dram.tile([BT, 128, OE], bf16, tag="cc_out",
                                   addr_space="Shared")
                nc.gpsimd.collective_compute(
                    "AllReduce", ALU.add,
                    replica_groups=[list(range(NCORES))],
                    ins=[cc_in.opt()], outs=[cc_out.opt()])
                nc.sync.dma_start(
                    out=sf[:], in_=cc_out.rearrange("b p f -> p b f"))

                # squash: v = s * sqrt(ss)/(1+ss) per (b, o); iteration 1
                # carries c=1/O as s_raw = O*s_true.
                sq = work.tile([128, BT, OE], f32, tag="sq")
                nc.vector.tensor_tensor(out=sq[:], in0=sf[:], in1=sf[:],
                                        op=ALU.mult)
                ss = work.tile([128, BT, O], f32, tag="ss")
                nc.vector.reduce_sum(
                    out=ss[:], in_=sq.rearrange("p b (o e) -> p b o e", o=O),
                    axis=mybir.AxisListType.X)
                t1 = work.tile([128, BT, O], f32, tag="t1")
                nc.scalar.activation(out=t1[:], in_=ss[:], func=AF.Sqrt)
                den = work.tile([128, BT, O], f32, tag="den")
                if first:
                    # ss_raw = O^2*ss_true:
                    #   v = s_raw*(1/O^2)*sqrt(ss_raw)/(1+ss_raw/O^2)
                    nc.vector.tensor_scalar(
                        out=den[:], in0=ss[:], scalar1=1.0 / (O * O),
                        scalar2=1.0, op0=ALU.mult, op1=ALU.add)
                else:
                    nc.vector.tensor_scalar_add(den[:], ss[:], 1.0)
                nc.vector.reciprocal(out=den[:], in_=den[:])
                rat = work.tile([128, BT, O], f32, tag="rat")
                nc.vector.tensor_tensor(out=rat[:], in0=t1[:], in1=den[:],
                                        op=ALU.mult)
                if first:
                    nc.vector.tensor_scalar_mul(rat[:], rat[:], 1.0 / (O * O))
                # prefetch Exp table for the next softmax; rides the
                # agreement-path slack
                nc.scalar.activation(out=dmy[:], in_=rat[:, 0, 0:1],
                                     func=AF.Exp)
                nc.vector.tensor_tensor(
                    out=vb4[:], in0=sf4[:],
                    in1=_bc(bass, rat[:], DOUT), op=ALU.mult)

                # G = x_flat^T @ v ; agree = (1/B) sum_de W*G ; b += agree.
                # Three (i,d)-tiles share one PSUM bank (3*640B < 2KB) so
                # the W*G multiply and e-reduction run once per group.
                GW = 3
                for g in range(0, NT, GW):
                    g_ps = ps_g.tile([128, GW, OE], f32, tag="g_ps")
                    for j in range(GW):
                        for bt in range(BT):
                            nc.tensor.matmul(
                                g_ps[:, j, :],
                                xf[:, bt, (g + j) * 128:(g + j + 1) * 128],
                                vb[:, bt, :],
                                start=(bt == 0), stop=(bt == BT - 1))
                    nc.vector.tensor_tensor(
                        out=WG[:, g:g + GW, :], in0=Wb[:, g:g + GW, :],
                        in1=g_ps[:], op=ALU.mult)
                    with nc.allow_low_precision("agreement tolerates bf16"):
                        nc.vector.reduce_sum(
                            out=A1[:, g:g + GW, :],
                            in_=WG[:, g:g + GW, :].rearrange(
                                "p g (o e) -> p (g o) e", o=O),
                            axis=mybir.AxisListType.X)
                # d-sums of all NT tiles land in one PSUM tile so the
                # b-update is a single vector op
                a_ps = ps_a.tile([128, NT, O], f32, tag="a_ps")
                for mt in range(NT):
                    nc.tensor.matmul(a_ps[:, mt, :], Mblk[:],
                                     A1[:, mt, :], start=True, stop=True)
                if first:
                    nc.vector.tensor_scalar_mul(bq[:], a_ps[:], 1.0 / B)
                else:
                    nc.vector.scalar_tensor_tensor(
                        out=bq[:], in0=a_ps[:], scalar=1.0 / B,
                        in1=bq[:], op0=ALU.mult, op1=ALU.add)

    nc.compile()
    return nc


def _get_nc():
    if "nc" not in _CACHE:
        _CACHE["nc"] = _build()
    return _CACHE["nc"]


def _tile128(a):
    """[R, C] -> [128, R//128, C] with row r = t*128+p at [p, t]."""
    r, c = a.shape
    return np.ascontiguousarray(
        a.reshape(r // 128, 128, c).transpose(1, 0, 2))


def _make_in_maps(x, W):
    from concourse import mybir
    bfdt = mybir.dt.np(mybir.dt.bfloat16)
    x = np.asarray(x, dtype=np.float32)
    W = np.asarray(W, dtype=np.float32)
    mblk = np.kron(np.eye(16, dtype=np.float32),
                   np.ones((8, 8), dtype=np.float32)).astype(bfdt)
    in_maps = []
    for core in range(NCORES):
        isl = slice(core * I_SH, (core + 1) * I_SH)
        x_flat = x[:, isl, :].reshape(B, ID)
        w_flat = W[isl].transpose(0, 2, 1, 3).reshape(ID, OE)
        in_maps.append({
            "xT": _tile128(np.ascontiguousarray(x_flat.T)).astype(bfdt),
            "xf": _tile128(x_flat).astype(bfdt),
            "Wb": _tile128(w_flat).astype(bfdt),
            "Mblk": mblk,
        })
    return in_maps


def _ensure_ntff_hook():
    """This image's antenv lacks axon_hooks; reconstruct it so trace=True
    can reach the NTFF profiler in libaxon_pjrt.so."""
    import sys
    import types
    try:
        import antenv.axon_hooks  # noqa: F401
        return
    except ImportError:
        pass
    try:
        import antenv
        from trn_agent_boot.trn_boot import _ntff_profile_via_ctypes
        hook = _ntff_profile_via_ctypes("/opt/axon/libaxon_pjrt.so")
        mod = types.ModuleType("antenv.axon_hooks")
        mod._hook = hook
        mod.get_axon_ntff_profile_hook = lambda: mod._hook
        mod.set_axon_ntff_profile_hook = (
            lambda h: setattr(mod, "_hook", h))
        sys.modules["antenv.axon_hooks"] = mod
        antenv.axon_hooks = mod
    except Exception as e:  # profiling is best-effort
        print("ntff hook setup failed:", e)


def _run_hw(x, W, trace=False, **kwargs):
    from concourse import bass_utils
    if trace:
        _ensure_ntff_hook()
    nc = _get_nc()
    res = bass_utils.run_bass_kernel_spmd(
        nc, _make_in_maps(x, W), core_ids=list(range(NCORES)),
        trace=trace, **kwargs)
    out = np.concatenate([res.results[c]["out"] for c in range(NCORES)],
                         axis=0)
    return out.reshape(B, O, DOUT)[..., None].astype(np.float32), res


def kernel(x, W):
    out, _ = _run_hw(x, W, trace=False)
    return out
